# revision 3
# baseline (speedup 1.0000x reference)
"""Trainium2 Bass kernel for CrossMotorFeatureExtractor.

x: (256, 24, 32768) fp32 -> (256, 14). Per sample (4 motors x 6T signal):
energy std/ratio + 6 Pearson corrs (need fp32-precision Gram: corr
denominators go down to ~4e-6) + 6 mean-abs-diffs via
sum|a-b| = 2*sum max(a,b) - S_a - S_b (fp16 is plenty).

Per core (32 samples, pure data parallel over 8 cores):
  - DMA (HWDGE) loads fp32 into col-major staging tiles.
  - ACT transposes to a time-major fp32 tile (packed matmul operands must be
    single-stride APs) and casts col-major fp32 -> fp16 for the DVE.
  - PE: Gram of [32 data cols + ones] per 8-sample group, packed 3
    time-chunks per matmul (99-col stationary == rhs, flat APs; the three
    33x33 diagonal blocks of the 99x99 PSUM accumulate the per-time-class
    Grams; host sums them). One matmul per 3 chunks beats the ~70ns/MM
    per-instruction floor.
  - DVE: fp16 tensor_max (2x_1P) for the 6 motor pairs, then a binary
    add-tree along time down to 6 partials per (pair,sample) column;
    partials accumulate into an fp32 SBUF tensor; host does the final
    cross-partition sum.
  - GPSIMD optionally takes the tree's first level to unload the DVE.
"""

import re

import numpy as np

import bass_rust
import concourse.bacc as bacc
import concourse.tile as tile
from concourse import mybir
import concourse.bass as bass
from concourse.bass_utils import run_bass_kernel_spmd
from concourse.vector_clock import ScopedClock

EPS = 1e-8
B, CH, T = 256, 24, 32768
NCORES = 8
BL = B // NCORES
SIG = 6 * T
P = 128
F = SIG // P  # 1536
SAMP_STRIDE = CH * T
MOT_STRIDE = SIG
PAIRS = [(0, 1), (0, 2), (0, 3), (1, 2), (1, 3), (2, 3)]
DIFF_PAIRS = [(0, 2), (1, 3), (0, 1), (1, 2), (2, 3), (3, 0)]
F32 = mybir.dt.float32
FP16 = mybir.dt.float16

GS = 8
NG = BL // GS  # 4 groups
TC = 192
NCH = F // TC  # 8 chunk-tiles per group
ND = 4 * GS  # 32 data cols
NC1 = ND + 1  # +ones = 33
PK = 3  # time-chunks packed per matmul
NPK = TC // PK
NMX = 6 * GS  # 48 max cols
TREE_H = [TC // 2, TC // 4, TC // 8, TC // 16, TC // 32]  # 96..6
NPART = TC // 32  # 6 partials per col
GP_TREE = True  # first tree level on gpsimd


def _drain_and_barrier_split(self, tick_clock, wait_clock):
    nc = self.nc
    gc = tick_clock.global_clock
    vals = eval(re.search(r"\[(.*)\]", repr(gc)).group(0))
    for i, v in enumerate(vals):
        if v:
            onehot = [0] * len(vals)
            onehot[i] = v
            nop = nc.sync.nop(nofuse=True, hint=f"drain_split_{i}")
            wait_clock.add_sem_waits(
                nop.ins, ScopedClock({None: bass_rust.VectorClock(onehot)})
            )
    nc.sync.drain()
    nc.all_engine_barrier()
    assert self.sems is not None
    popped = nc._tile_sem_poison_stack.pop()
    assert popped is self._sem_poison
    nc.clear_and_free_semaphores(list(self.sems.allocated().values()))
    nc.all_engine_barrier()


tile.TileContext._drain_and_barrier = _drain_and_barrier_split


def _build(reps: int = 1, gp_tree: bool = GP_TREE):
    nc = bacc.Bacc(None, target_bir_lowering=False)
    x = nc.dram_tensor("x", [BL, CH, T], F32, kind="ExternalInput")
    gram_out = nc.dram_tensor("gram", [3 * NC1, NG * 3 * NC1], F32,
                              kind="ExternalOutput")
    macc_out = nc.dram_tensor("macc", [P, NG * NMX * NPART], F32,
                              kind="ExternalOutput")

    with tile.TileContext(nc) as tcx:
        rep_loop = tcx.For_i(0, reps, 1) if reps > 1 else None
        if rep_loop is not None:
            rep_loop.__enter__()
        with (
            tcx.tile_pool(name="cmaj", bufs=2) as cmaj_pool,
            tcx.tile_pool(name="tmaj", bufs=2) as tmaj_pool,
            tcx.tile_pool(name="f16", bufs=2) as f16_pool,
            tcx.tile_pool(name="mx", bufs=2) as mx_pool,
            tcx.tile_pool(name="persist", bufs=1) as per_pool,
            tcx.tile_pool(name="psum", bufs=1, space="PSUM") as psum_pool,
        ):
            acc = per_pool.tile([P, NG * NMX * NPART], F32, tag="acc")
            gram_sb = per_pool.tile([P, NG * 3 * NC1], F32, tag="gram_sb")
            psum_g = [
                psum_pool.tile([P, 128], F32, tag=f"ps{g}", name=f"ps{g}")
                for g in range(NG)
            ]
            nc.vector.memset(acc[:, :], 0.0)

            for g in range(NG):
                for c in range(NCH):
                    cm = cmaj_pool.tile([P, NC1, TC], F32, tag="cm", name="cm")
                    tm = tmaj_pool.tile([P, TC * NC1], F32, tag="tm", name="tm")
                    f16 = f16_pool.tile([P, ND, TC], FP16, tag="f16", name="f16")
                    mx = mx_pool.tile([P, NMX, TC], FP16, tag="mx", name="mx")
                    # tree ping-pong scratch aliases the f16 tile (dead after
                    # the max pass; same-engine program order keeps it safe)
                    sc = (
                        f16[:, 0 : NMX // 2, :]
                        .rearrange("p a b -> p (a b)")
                        .rearrange("p (c q) -> p c q", q=TC // 2)
                    )
                    for sl in range(GS):
                        s_abs = g * GS + sl
                        src = bass.AP(
                            x,
                            s_abs * SAMP_STRIDE + c * TC,
                            [[F, P], [MOT_STRIDE, 4], [1, TC]],
                        )
                        # alternate the two HWDGE rings (qSP / qAct): one
                        # ring's descriptor feed rate caps at ~220 GB/s
                        eng = nc.scalar if sl % 2 else nc.sync
                        eng.dma_start(out=cm[:, 4 * sl : 4 * sl + 4, :],
                                      in_=src)
                    nc.vector.memset(cm[:, ND, :], 1.0)
                    # time-major fp32 for the packed Gram matmuls
                    nc.scalar.copy(
                        out=tm[:, :].rearrange("p (t c) -> p t c", c=NC1),
                        in_=cm[:, :, :].rearrange("p c t -> p t c"),
                    )
                    # fp16 copy for the DVE max pass
                    nc.scalar.copy(out=f16[:, :, :], in_=cm[:, 0:ND, :])
                    # 6 pairwise maxes (2x_1P)
                    for pidx, (i, j) in enumerate(PAIRS):
                        nc.vector.tensor_max(
                            out=mx[:, GS * pidx : GS * pidx + GS, :],
                            in0=f16[:, i:ND:4, :],
                            in1=f16[:, j:ND:4, :],
                        )
                    # add-tree down to NPART partials per col (ping-pong mx<->sc)
                    h = TC // 2
                    eng = nc.gpsimd if gp_tree else nc.vector
                    eng.tensor_add(out=sc[:, :, 0:h], in0=mx[:, :, 0:h],
                                   in1=mx[:, :, h : 2 * h])
                    cur, oth, h = sc, mx, h // 2
                    while h >= NPART:
                        nc.vector.tensor_add(
                            out=oth[:, :, 0:h], in0=cur[:, :, 0:h],
                            in1=cur[:, :, h : 2 * h],
                        )
                        cur, oth, h = oth, cur, h // 2
                    a0 = (g * NMX + 0) * NPART
                    nc.vector.tensor_add(
                        out=acc[:, a0 : a0 + NMX * NPART].rearrange(
                            "p (c q) -> p c q", q=NPART),
                        in0=acc[:, a0 : a0 + NMX * NPART].rearrange(
                            "p (c q) -> p c q", q=NPART),
                        in1=cur[:, :, 0:NPART],
                    )
                    # packed Gram matmuls: 99-col stationary == rhs, flat APs
                    for m in range(NPK):
                        op = tm[:, m * 3 * NC1 : (m + 1) * 3 * NC1]
                        nc.tensor.matmul(
                            out=psum_g[g][: 3 * NC1, : 3 * NC1],
                            lhsT=op,
                            rhs=op,
                            start=(c == 0 and m == 0),
                            stop=(c == NCH - 1 and m == NPK - 1),
                        )
                nc.scalar.copy(
                    out=gram_sb[: 3 * NC1, g * 3 * NC1 : (g + 1) * 3 * NC1],
                    in_=psum_g[g][: 3 * NC1, : 3 * NC1],
                )
            nc.sync.dma_start(out=gram_out[:, :],
                              in_=gram_sb[: 3 * NC1, :])
            nc.sync.dma_start(out=macc_out[:, :], in_=acc[:, :])
        if rep_loop is not None:
            rep_loop.__exit__(None, None, None)

    nc.finalize()
    return nc


_NC = None
_LAST_RES = None


def _decode(grams, maccs):
    out = np.zeros((B, 14), dtype=np.float64)
    for k in range(NCORES):
        gram = grams[k]  # (99, NG*99)
        macc = maccs[k]  # (128, NG*NMX*NPART)
        for g in range(NG):
            Gb = gram[:, g * 3 * NC1 : (g + 1) * 3 * NC1]
            G = sum(
                Gb[t * NC1 : (t + 1) * NC1, t * NC1 : (t + 1) * NC1]
                for t in range(3)
            )
            ma = macc[:, g * NMX * NPART : (g + 1) * NMX * NPART]
            msums = ma.reshape(P, NMX, NPART).sum(axis=(0, 2))  # per col
            for sl in range(GS):
                b = k * BL + g * GS + sl
                c0 = 4 * sl
                S = G[ND, c0 : c0 + 4]
                Gm = G[c0 : c0 + 4, c0 : c0 + 4]
                Q = np.diag(Gm)
                energies = Q / SIG
                e_std = np.std(energies, ddof=1)
                e_ratio = energies.max() / (energies.min() + EPS)
                Cm = Gm - np.outer(S, S) / SIG
                norms = np.sqrt(np.diag(Cm))
                corrs = [Cm[i, j] / (norms[i] * norms[j] + EPS)
                         for i, j in PAIRS]
                msum = {pair: msums[GS * pidx + sl]
                        for pidx, pair in enumerate(PAIRS)}
                diffs = []
                for i, j in DIFF_PAIRS:
                    key = (i, j) if (i, j) in msum else (j, i)
                    diffs.append((2.0 * msum[key] - S[i] - S[j]) / SIG)
                out[b] = [e_std, e_ratio, *corrs, *diffs]
    return out.astype(np.float32)


def kernel(x: np.ndarray) -> np.ndarray:
    global _NC, _LAST_RES
    if _NC is None:
        _NC = _build()
    x = np.ascontiguousarray(x, dtype=np.float32)
    shards = x.reshape(NCORES, BL, CH, T)
    in_maps = [{"x": shards[k]} for k in range(NCORES)]
    res = run_bass_kernel_spmd(_NC, in_maps, core_ids=list(range(NCORES)))
    _LAST_RES = res
    grams = [res.results[k]["gram"].astype(np.float64) for k in range(NCORES)]
    maccs = [res.results[k]["macc"].astype(np.float64) for k in range(NCORES)]
    return _decode(grams, maccs)


# revision 5
# speedup vs baseline: 1.2654x; 1.2654x over previous
"""Trainium2 Bass kernel for CrossMotorFeatureExtractor.

x: (256, 24, 32768) fp32 -> (256, 14). Per sample (4 motors x 6T signal):
energy std/ratio + 6 Pearson corrs (need fp32-precision Gram: corr
denominators go down to ~4e-6) + 6 mean-abs-diffs via
sum|a-b| = 2*sum max(a,b) - S_a - S_b (fp16 is plenty).

Per core (32 samples, pure data parallel over 8 cores):
  - DMA (HWDGE) loads fp32 into col-major staging tiles.
  - ACT transposes to a time-major fp32 tile (packed matmul operands must be
    single-stride APs) and casts col-major fp32 -> fp16 for the DVE.
  - PE: Gram of [32 data cols + ones] per 8-sample group, packed 3
    time-chunks per matmul (99-col stationary == rhs, flat APs; the three
    33x33 diagonal blocks of the 99x99 PSUM accumulate the per-time-class
    Grams; host sums them). One matmul per 3 chunks beats the ~70ns/MM
    per-instruction floor.
  - DVE: fp16 tensor_max (2x_1P) for the 6 motor pairs, then a binary
    add-tree along time down to 6 partials per (pair,sample) column;
    partials accumulate into an fp32 SBUF tensor; host does the final
    cross-partition sum.
  - GPSIMD optionally takes the tree's first level to unload the DVE.
"""

import re

import numpy as np

import bass_rust
import concourse.bacc as bacc
import concourse.tile as tile
from concourse import mybir
import concourse.bass as bass
from concourse.bass_utils import run_bass_kernel_spmd
from concourse.vector_clock import ScopedClock

EPS = 1e-8
B, CH, T = 256, 24, 32768
NCORES = 8
BL = B // NCORES
SIG = 6 * T
P = 128
F = SIG // P  # 1536
SAMP_STRIDE = CH * T
MOT_STRIDE = SIG
PAIRS = [(0, 1), (0, 2), (0, 3), (1, 2), (1, 3), (2, 3)]
DIFF_PAIRS = [(0, 2), (1, 3), (0, 1), (1, 2), (2, 3), (3, 0)]
F32 = mybir.dt.float32
FP16 = mybir.dt.float16

GS = 8
NG = BL // GS  # 4 groups
TC = 128  # compute chunk (time elems per partition per compute tile)
TC_DMA = 384  # DMA staging chunk: 1.5KB descriptor runs (426 GB/s vs
#               218 GB/s at 768B — single qSP HWDGE ring feed-rate bound)
CPD = TC_DMA // TC  # compute chunks per DMA tile
NCH = F // TC  # 12 compute chunks per group
NDMA = F // TC_DMA  # 4 DMA tiles per group
ND = 4 * GS  # 32 data cols
NC1 = ND + 1  # +ones = 33
PK = 3  # time-chunks packed per matmul
NPK = TC // PK  # 42 full 3-packs; remainder handled as one 2-pack
NMX = 6 * GS  # 48 max cols
NPART = TC // 32  # 4 partials per col after the add-tree
GP_TREE = True  # first tree level on gpsimd


def _drain_and_barrier_split(self, tick_clock, wait_clock):
    nc = self.nc
    gc = tick_clock.global_clock
    vals = eval(re.search(r"\[(.*)\]", repr(gc)).group(0))
    for i, v in enumerate(vals):
        if v:
            onehot = [0] * len(vals)
            onehot[i] = v
            nop = nc.sync.nop(nofuse=True, hint=f"drain_split_{i}")
            wait_clock.add_sem_waits(
                nop.ins, ScopedClock({None: bass_rust.VectorClock(onehot)})
            )
    nc.sync.drain()
    nc.all_engine_barrier()
    assert self.sems is not None
    popped = nc._tile_sem_poison_stack.pop()
    assert popped is self._sem_poison
    nc.clear_and_free_semaphores(list(self.sems.allocated().values()))
    nc.all_engine_barrier()


tile.TileContext._drain_and_barrier = _drain_and_barrier_split


def _build(reps: int = 1, gp_tree: bool = GP_TREE):
    nc = bacc.Bacc(None, target_bir_lowering=False)
    x = nc.dram_tensor("x", [BL, CH, T], F32, kind="ExternalInput")
    gram_out = nc.dram_tensor("gram", [3 * NC1, NG * 3 * NC1], F32,
                              kind="ExternalOutput")
    macc_out = nc.dram_tensor("macc", [P, NG * NMX * NPART], F32,
                              kind="ExternalOutput")

    with tile.TileContext(nc) as tcx:
        rep_loop = tcx.For_i(0, reps, 1) if reps > 1 else None
        if rep_loop is not None:
            rep_loop.__enter__()
        with (
            tcx.tile_pool(name="cmaj", bufs=2) as cmaj_pool,
            tcx.tile_pool(name="tmaj", bufs=2) as tmaj_pool,
            tcx.tile_pool(name="f16", bufs=2) as f16_pool,
            tcx.tile_pool(name="mx", bufs=2) as mx_pool,
            tcx.tile_pool(name="persist", bufs=1) as per_pool,
            tcx.tile_pool(name="psum", bufs=1, space="PSUM") as psum_pool,
        ):
            acc = per_pool.tile([P, NG * NMX * NPART], F32, tag="acc")
            gram_sb = per_pool.tile([P, NG * 3 * NC1], F32, tag="gram_sb")
            psum_g = [
                psum_pool.tile([P, 128], F32, tag=f"ps{g}", name=f"ps{g}")
                for g in range(NG)
            ]
            nc.vector.memset(acc[:, :], 0.0)

            for g in range(NG):
                for cd in range(NDMA):
                    cm = cmaj_pool.tile([P, ND, TC_DMA], F32, tag="cm",
                                        name="cm")
                    for sl in range(GS):
                        s_abs = g * GS + sl
                        src = bass.AP(
                            x,
                            s_abs * SAMP_STRIDE + cd * TC_DMA,
                            [[F, P], [MOT_STRIDE, 4], [1, TC_DMA]],
                        )
                        nc.sync.dma_start(out=cm[:, 4 * sl : 4 * sl + 4, :],
                                          in_=src)
                    for cc in range(CPD):
                        c = cd * CPD + cc
                        cmv = cm[:, :, cc * TC : (cc + 1) * TC]
                        tm = tmaj_pool.tile([P, TC * NC1], F32, tag="tm",
                                            name="tm")
                        f16 = f16_pool.tile([P, ND, TC], FP16, tag="f16",
                                            name="f16")
                        mx = mx_pool.tile([P, NMX, TC], FP16, tag="mx",
                                          name="mx")
                        # tree ping-pong scratch aliases the f16 tile (dead
                        # after the max pass; DVE program order keeps it safe)
                        sc = (
                            f16[:, 0 : NMX // 2, :]
                            .rearrange("p a b -> p (a b)")
                            .rearrange("p (c q) -> p c q", q=TC // 2)
                        )
                        # ones column lives at (t*33 + 32) of the time-major
                        nc.vector.memset(tm[:, ND :: NC1], 1.0)
                        # time-major fp32 data cols for the packed Gram
                        nc.scalar.copy(
                            out=tm[:, :]
                            .rearrange("p (t c) -> p t c", c=NC1)[:, :, 0:ND],
                            in_=cmv.rearrange("p c t -> p t c"),
                        )
                        # fp16 copy for the DVE max pass
                        nc.scalar.copy(out=f16[:, :, :], in_=cmv)
                        # 6 pairwise maxes (2x_1P)
                        for pidx, (i, j) in enumerate(PAIRS):
                            nc.vector.tensor_max(
                                out=mx[:, GS * pidx : GS * pidx + GS, :],
                                in0=f16[:, i:ND:4, :],
                                in1=f16[:, j:ND:4, :],
                            )
                        # add-tree to NPART partials per col (ping-pong)
                        h = TC // 2
                        eng = nc.gpsimd if gp_tree else nc.vector
                        eng.tensor_add(out=sc[:, :, 0:h], in0=mx[:, :, 0:h],
                                       in1=mx[:, :, h : 2 * h])
                        cur, oth, h = sc, mx, h // 2
                        while h >= NPART:
                            nc.vector.tensor_add(
                                out=oth[:, :, 0:h], in0=cur[:, :, 0:h],
                                in1=cur[:, :, h : 2 * h],
                            )
                            cur, oth, h = oth, cur, h // 2
                        a0 = g * NMX * NPART
                        nc.vector.tensor_add(
                            out=acc[:, a0 : a0 + NMX * NPART].rearrange(
                                "p (c q) -> p c q", q=NPART),
                            in0=acc[:, a0 : a0 + NMX * NPART].rearrange(
                                "p (c q) -> p c q", q=NPART),
                            in1=cur[:, :, 0:NPART],
                        )
                        # packed Gram matmuls: 3 chunks per 99-col stationary
                        # == rhs (flat APs); trailing 2-pack covers TC%3
                        packs = [(m * 3, 3) for m in range(TC // 3)]
                        if TC % 3:
                            packs.append((TC - TC % 3, TC % 3))
                        for pi, (t0, pk) in enumerate(packs):
                            op = tm[:, t0 * NC1 : (t0 + pk) * NC1]
                            nc.tensor.matmul(
                                out=psum_g[g][: pk * NC1, : pk * NC1],
                                lhsT=op,
                                rhs=op,
                                start=(c == 0 and pi == 0),
                                stop=(c == NCH - 1 and pi == len(packs) - 1),
                            )
                nc.scalar.copy(
                    out=gram_sb[: 3 * NC1, g * 3 * NC1 : (g + 1) * 3 * NC1],
                    in_=psum_g[g][: 3 * NC1, : 3 * NC1],
                )
            nc.sync.dma_start(out=gram_out[:, :],
                              in_=gram_sb[: 3 * NC1, :])
            nc.sync.dma_start(out=macc_out[:, :], in_=acc[:, :])
        if rep_loop is not None:
            rep_loop.__exit__(None, None, None)

    nc.finalize()
    return nc


_NC = None
_LAST_RES = None


def _decode(grams, maccs):
    out = np.zeros((B, 14), dtype=np.float64)
    for k in range(NCORES):
        gram = grams[k]  # (99, NG*99)
        macc = maccs[k]  # (128, NG*NMX*NPART)
        for g in range(NG):
            Gb = gram[:, g * 3 * NC1 : (g + 1) * 3 * NC1]
            G = sum(
                Gb[t * NC1 : (t + 1) * NC1, t * NC1 : (t + 1) * NC1]
                for t in range(3)
            )
            ma = macc[:, g * NMX * NPART : (g + 1) * NMX * NPART]
            msums = ma.reshape(P, NMX, NPART).sum(axis=(0, 2))  # per col
            for sl in range(GS):
                b = k * BL + g * GS + sl
                c0 = 4 * sl
                S = G[ND, c0 : c0 + 4]
                Gm = G[c0 : c0 + 4, c0 : c0 + 4]
                Q = np.diag(Gm)
                energies = Q / SIG
                e_std = np.std(energies, ddof=1)
                e_ratio = energies.max() / (energies.min() + EPS)
                Cm = Gm - np.outer(S, S) / SIG
                norms = np.sqrt(np.diag(Cm))
                corrs = [Cm[i, j] / (norms[i] * norms[j] + EPS)
                         for i, j in PAIRS]
                msum = {pair: msums[GS * pidx + sl]
                        for pidx, pair in enumerate(PAIRS)}
                diffs = []
                for i, j in DIFF_PAIRS:
                    key = (i, j) if (i, j) in msum else (j, i)
                    diffs.append((2.0 * msum[key] - S[i] - S[j]) / SIG)
                out[b] = [e_std, e_ratio, *corrs, *diffs]
    return out.astype(np.float32)


def kernel(x: np.ndarray) -> np.ndarray:
    global _NC, _LAST_RES
    if _NC is None:
        _NC = _build()
    x = np.ascontiguousarray(x, dtype=np.float32)
    shards = x.reshape(NCORES, BL, CH, T)
    in_maps = [{"x": shards[k]} for k in range(NCORES)]
    res = run_bass_kernel_spmd(_NC, in_maps, core_ids=list(range(NCORES)))
    _LAST_RES = res
    grams = [res.results[k]["gram"].astype(np.float64) for k in range(NCORES)]
    maccs = [res.results[k]["macc"].astype(np.float64) for k in range(NCORES)]
    return _decode(grams, maccs)
